# revision 1
# baseline (speedup 1.0000x reference)
"""ADMM-Net (DiffuserCam-style ADMM deconvolution) on 8 TRN2 NeuronCores.

Data-parallel: one 512x512 image per core, 20 ADMM iterations on the padded
1024x1024 grid. All fftshifts cancel algebraically; each iteration needs
2 real forward 2D-FFTs (of s_hat, Z) and 1 packed complex inverse 2D-FFT
that yields X' (real part) and H(X') (imag part) simultaneously. FFTs are
DFT-by-matmul on the TensorEngine in float32r (full-rate fp32).

Scaled state (A1=a1/MU1, A2=a2/MU2, A3=a3/MU3, HX=H(X) carried):
  W2d = soft(psi_d(X)+A2d, TAU/MU2) - A2d
  T3  = max(A3+X,0) - A3
  u   = A1 + HX + pad(y)/MU1;  Z = MU1*Vdiv*u - A1
        (MU1*Vdiv = 1 outside center, MU1/(1+MU1) inside => Z=HX outside)
  s^  = T3 + psit(W2)
  G   = P1*F(s^) + P2*F(Z)   [P1=(1+iHf)*MU3*Rdiv, P2=(1+iHf)*MU1*Rdiv*conj(Hf),
                              1/N^2 inverse scale folded in, stored transposed]
  X' + i*HX' = IFFT2(G);  A1'=HX'-Z, A2d'=psi_d(X')-W2d, A3'=X'-T3
Output: X[256:768, 256:768] after 20 iterations.
"""
import numpy as np

import concourse.bass as bass
import concourse.bacc as bacc
import concourse.mybir as mybir
import concourse.tile as tile
from concourse.bass_utils import run_bass_kernel_spmd

F32 = mybir.dt.float32
F32R = mybir.dt.float32r
AF = mybir.ActivationFunctionType
ALU = mybir.AluOpType

D = 1024
MU1, MU2, MU3 = 1e-6, 1e-5, 1e-5
TAU = 1e-4 * 1000.0
CTHR = TAU / MU2          # 1e4
VCEN = MU1 / (1.0 + MU1)  # center factor of MU1*Vdiv
ALPHA = 295.6  # ~rms|Hf| for h~U[0,1]: balances packed inverse (HX stored /ALPHA)
ITERS = 20

_CACHE = {}


def _build(iters=ITERS, use_loop=True, debug=False):
    nc = bacc.Bacc()

    ys_p = nc.declare_dram_parameter("ys", [512, 512], F32, isOutput=False)
    c_p = nc.declare_dram_parameter("cmat", [D, D], F32R, isOutput=False)
    s_p = nc.declare_dram_parameter("smat", [D, D], F32R, isOutput=False)
    id_p = nc.declare_dram_parameter("idp", [128, 128], F32R, isOutput=False)
    p1r_p = nc.declare_dram_parameter("p1r", [D, D], F32, isOutput=False)
    p1i_p = nc.declare_dram_parameter("p1i", [D, D], F32, isOutput=False)
    p2r_p = nc.declare_dram_parameter("p2r", [D, D], F32, isOutput=False)
    p2i_p = nc.declare_dram_parameter("p2i", [D, D], F32, isOutput=False)
    out_p = nc.declare_dram_parameter("out", [512, 512], F32, isOutput=True)
    dbg_p = {}
    if debug:
        for nm in ("Z", "SH", "GR", "GI", "X", "HX"):
            dbg_p[nm] = nc.declare_dram_parameter("dbg_" + nm, [D, D], F32, isOutput=True)

    dt_ = {}
    for nm in ("X", "HX", "W20", "W21", "T3", "FZR", "FZI"):
        dt_[nm] = nc.dram_tensor(nm, [D, D], F32)
    for nm in ("SH", "Z", "GR", "GI"):
        dt_[nm] = nc.dram_tensor(nm, [D, D], F32R)

    with tile.TileContext(nc) as tc:
        with (
            tc.tile_pool(name="consts", bufs=1) as cpool,
            tc.tile_pool(name="ld", bufs=2) as ld,     # loads (halves + xtf full)
            tc.tile_pool(name="tmp", bufs=2) as tmp,   # elementwise temps (halves)
            tc.tile_pool(name="wsrc", bufs=3) as wsrc,  # [128,128] lhsT data chunks
            tc.tile_pool(name="tt", bufs=2) as ttp,      # [128,512] drain temps
            tc.tile_pool(name="t1t", bufs=8) as t1t,     # inter-stage tiles
            tc.tile_pool(name="pmm", bufs=3, space="PSUM") as pmm,
            tc.tile_pool(name="pmc", bufs=2, space="PSUM") as pmc,
        ):
            CT = cpool.tile([128, 8 * D], F32R, tag="CT", name="CT")
            ST = cpool.tile([128, 8 * D], F32R, tag="ST", name="ST")
            nc.sync.dma_start(
                CT[:].rearrange("p (j k) -> p j k", j=8),
                c_p.rearrange("(j p) k -> p j k", p=128),
            )
            nc.sync.dma_start(
                ST[:].rearrange("p (j k) -> p j k", j=8),
                s_p.rearrange("(j p) k -> p j k", p=128),
            )

            def cs_chunk(mat, j, mi):
                return mat[:, j * D + mi * 128: j * D + (mi + 1) * 128]

            V = nc.vector
            SC = nc.scalar
            GP = nc.gpsimd

            # ---- zero-init state ----
            zz = cpool.tile([128, 512], F32, tag="zz", name="zz")
            V.memset(zz[:], 0.0)
            nct = cpool.tile([128, 1], F32, tag="nct", name="nct")
            V.memset(nct[:], -CTHR)
            for nm in ("X", "HX", "W20", "W21", "T3", "Z"):
                for mi in range(8):
                    for hi2 in range(2):
                        nc.sync.dma_start(
                            dt_[nm][mi * 128:(mi + 1) * 128,
                                    hi2 * 512:(hi2 + 1) * 512].bitcast(F32)
                            if nm == "Z" else
                            dt_[nm][mi * 128:(mi + 1) * 128,
                                    hi2 * 512:(hi2 + 1) * 512], zz[:])

            def ldh(dram, mi, hi, tag, shift=0, dtype=F32, cast32=False, eng=None):
                """Load a [128,512] half-tile (rows mi*128 + shift, cols hi*512)."""
                t = ld.tile([128, 512], dtype, tag=tag, name=tag)
                eng = eng or nc.sync
                r0 = mi * 128 + shift
                cs = slice(hi * 512, (hi + 1) * 512)
                src = dram[r0:r0 + 128, cs] if 0 <= r0 and r0 + 128 <= D else None
                if shift == 0:
                    src = dram[r0:r0 + 128, cs]
                    eng.dma_start(t[:], src.bitcast(F32) if cast32 else src)
                elif shift == -1:
                    if mi == 0:
                        eng.dma_start(t[0:1, :], dram[D - 1:D, cs])
                        eng.dma_start(t[1:128, :], dram[0:127, cs])
                    else:
                        eng.dma_start(t[:], dram[r0:r0 + 128, cs])
                elif shift == 1:
                    if mi == 7:
                        eng.dma_start(t[0:127, :], dram[r0:D, cs])
                        eng.dma_start(t[127:128, :], dram[0:1, cs])
                    else:
                        eng.dma_start(t[:], dram[r0:r0 + 128, cs])
                return t

            def ldf(dram, mi, tag="xtf"):
                t = ld.tile([128, D], F32, tag=tag, name=tag)
                nc.sync.dma_start(t[:], dram[mi * 128:(mi + 1) * 128, :])
                return t

            def sth(dram, mi, hi, t):
                nc.sync.dma_start(
                    dram[mi * 128:(mi + 1) * 128, hi * 512:(hi + 1) * 512], t[:])

            def th(tag, dtype=F32):
                return tmp.tile([128, 512], dtype, tag=tag, name=tag)

            # Chunk order: mi=0 needs the wrap row (1023) so it must run
            # last; 1..7 pipeline behind the producer's emit order.
            ORDER = [1, 2, 3, 4, 5, 6, 7, 0]

            # ------------- fused end-of-iteration sweep -------------
            # from (X', HX~', W2, T3, Z) compute next (W2, T3, Z) in place.
            def phase_fused():
                for mi in ORDER:
                    xtf = ldf(dt_["X"], mi)
                    for hi in range(2):
                        hs = hi * 512
                        xh = xtf[:, hs:hs + 512]
                        xup = ldh(dt_["X"], mi, hi, "l1", shift=-1)
                        hxt = ldh(dt_["HX"], mi, hi, "l2")
                        w0o = ldh(dt_["W20"], mi, hi, "l4")
                        w1o = ldh(dt_["W21"], mi, hi, "l6")
                        t3o = ldh(dt_["T3"], mi, hi, "m")

                        # Z' first: it gates the next forward FFT's stage 1.
                        # Z' = alpha*HX~' outside; center: VCEN*u' - alpha*HX~' + Z
                        z = th("z", F32R)
                        V.tensor_scalar_mul(z[:], hxt[:], ALPHA)
                        if 2 <= mi <= 5:
                            zold = ldh(dt_["Z"], mi, hi, "l3", cast32=True)
                            lo, hicol = (256, 512) if hi == 0 else (0, 256)
                            yt = ld.tile([128, 256], F32, tag="yt", name="yt")
                            yc0 = hi * 512 + lo - 256
                            nc.sync.dma_start(
                                yt[:],
                                ys_p[(mi - 2) * 128:(mi - 1) * 128, yc0:yc0 + 256])
                            uc = th("u")
                            V.scalar_tensor_tensor(
                                uc[:, 0:256], hxt[:, lo:hicol], 2.0 * ALPHA,
                                zold[:, lo:hicol], ALU.mult, ALU.subtract)
                            V.tensor_add(uc[:, 0:256], uc[:, 0:256], yt[:])
                            a1c = th("tb")
                            V.scalar_tensor_tensor(
                                a1c[:, 0:256], hxt[:, lo:hicol], ALPHA,
                                zold[:, lo:hicol], ALU.mult, ALU.subtract)
                            V.scalar_tensor_tensor(
                                z[:, lo:hicol], uc[:, 0:256], VCEN, a1c[:, 0:256],
                                ALU.mult, ALU.subtract)
                        sth(dt_["Z"], mi, hi, z)

                        # ch0: q = 2*psi0 - w0o ; w0' = soft(q) - psi0 + w0o
                        p = th("p")
                        V.tensor_sub(p[:], xup[:], xh)
                        q = th("ta")
                        V.scalar_tensor_tensor(q[:], p[:], 2.0, w0o[:],
                                               ALU.mult, ALU.subtract)
                        ta = th("tb")
                        V.tensor_scalar(ta[:], q[:], CTHR, 0.0, ALU.subtract, ALU.max)
                        tb = th("u")
                        SC.activation(tb[:], q[:], AF.Relu, bias=nct[:], scale=-1.0)
                        V.tensor_sub(ta[:], ta[:], tb[:])
                        V.tensor_sub(ta[:], ta[:], p[:])
                        V.tensor_add(ta[:], ta[:], w0o[:])
                        sth(dt_["W20"], mi, hi, ta)

                        # ch1
                        p1 = th("p")
                        if hi == 0:
                            V.tensor_sub(p1[:, 1:], xtf[:, 0:511], xtf[:, 1:512])
                            V.tensor_sub(p1[:, 0:1], xtf[:, D - 1:D], xtf[:, 0:1])
                        else:
                            V.tensor_sub(p1[:], xtf[:, 511:1023], xtf[:, 512:1024])
                        q1 = th("ta")
                        V.scalar_tensor_tensor(q1[:], p1[:], 2.0, w1o[:],
                                               ALU.mult, ALU.subtract)
                        ta1 = th("tb")
                        V.tensor_scalar(ta1[:], q1[:], CTHR, 0.0, ALU.subtract, ALU.max)
                        tb1 = th("u")
                        SC.activation(tb1[:], q1[:], AF.Relu, bias=nct[:], scale=-1.0)
                        V.tensor_sub(ta1[:], ta1[:], tb1[:])
                        V.tensor_sub(ta1[:], ta1[:], p1[:])
                        V.tensor_add(ta1[:], ta1[:], w1o[:])
                        sth(dt_["W21"], mi, hi, ta1)

                        # T3' = max(2X'-T3,0) - X' + T3
                        m = th("ta")
                        V.scalar_tensor_tensor(m[:], xh, 2.0, t3o[:],
                                               ALU.mult, ALU.subtract)
                        V.tensor_scalar_max(m[:], m[:], 0.0)
                        GP.tensor_tensor(m[:], m[:], xh, ALU.subtract)
                        GP.tensor_tensor(m[:], m[:], t3o[:], ALU.add)
                        sth(dt_["T3"], mi, hi, m)

            # ---------------- phase A2: s^ = T3 + psit(W2) ----------------
            # Entirely on GpSimd: keeps the Vector queue free for the FFT
            # psum drains, so SH is ready before fwd(SH) stage 1 needs it.
            def phase_a2():
                for mi in ORDER:
                    w1f = ldf(dt_["W21"], mi)
                    for hi in range(2):
                        w0d = ldh(dt_["W20"], mi, hi, "l1", shift=1)
                        w0 = ldh(dt_["W20"], mi, hi, "l2")
                        t3 = ldh(dt_["T3"], mi, hi, "l3")
                        dt0 = th("p")
                        GP.tensor_tensor(dt0[:], w0d[:], w0[:], ALU.subtract)
                        e = th("ta")
                        if hi == 0:
                            GP.tensor_tensor(e[:], w1f[:, 1:513],
                                             w1f[:, 0:512], ALU.subtract)
                        else:
                            GP.tensor_tensor(e[:, 0:511], w1f[:, 513:1024],
                                             w1f[:, 512:1023], ALU.subtract)
                            GP.tensor_tensor(e[:, 511:512], w1f[:, 0:1],
                                             w1f[:, D - 1:D], ALU.subtract)
                        GP.tensor_tensor(dt0[:], dt0[:], e[:], ALU.add)
                        sh = th("z", F32R)
                        GP.tensor_tensor(sh[:], dt0[:], t3[:], ALU.add)
                        sth(dt_["SH"], mi, hi, sh)

            # ---------------- generic FFT pass (data-as-weights) ----------------
            # Every stage: out = lhsT.T @ rhs with lhsT = 128x128 DATA chunk
            # (reused for 4 matmuls) and rhs = resident C/S rows [128,512].
            # Each stage transposes its input, so two stages per transform
            # give the correct orientation. Spectra land as [k0, k1].

            def rhs_cs(mat, j, half):
                return mat[:, j * D + half * 512: j * D + (half + 1) * 512]

            def real_stage(src_dram, outa, outb, jorder):
                """t1r/t1i tiles [cblk][c(128part), k0(1024)]:
                outa[cblk] = (x.T C)[cblk], outb = (x.T S)[cblk].
                Weight chunks load per row-block j (split DMAs) and the PSUM
                accumulation follows jorder, so stage 1 consumes the
                producer's row-chunks as they land in DRAM."""
                for cblk in range(8):
                    wt = wsrc.tile([128, 8 * 128], F32R, tag="w", name="w")
                    for j in jorder:
                        nc.sync.dma_start(
                            wt[:, j * 128:(j + 1) * 128],
                            src_dram[j * 128:(j + 1) * 128,
                                     cblk * 128:(cblk + 1) * 128])
                    ws = [wt[:, j * 128:(j + 1) * 128] for j in range(8)]
                    pA = [pmm.tile([128, 512], F32, tag="pp", name="pp") for _ in range(2)]
                    pB = [pmm.tile([128, 512], F32, tag="pn", name="pn") for _ in range(2)]
                    for idx, j in enumerate(jorder):
                        st, sp = (idx == 0), (idx == 7)
                        for hh in range(2):
                            nc.tensor.matmul(pA[hh][:], ws[j], rhs_cs(CT, j, hh),
                                             start=st, stop=sp)
                            nc.tensor.matmul(pB[hh][:], ws[j], rhs_cs(ST, j, hh),
                                             start=st, stop=sp)
                    for hh in range(2):
                        SC.copy(outa[cblk][:, hh * 512:(hh + 1) * 512], pA[hh][:])
                        SC.copy(outb[cblk][:, hh * 512:(hh + 1) * 512], pB[hh][:])

            def cplx_stage(a_t, b_t, out_emit, sigma, from_dram=False, dst=None):
                """inputs a,b: 8 tiles [128,1024] (SBUF) or dram tensors.
                Emits per (blk, half): R = (a.T C - b.T S), I = sigma*(b.T C + a.T S)
                via out_emit(blk, half, r_tile_sbuf, i_tile_sbuf)."""
                for blk in range(8):
                    if from_dram:
                        wta = wsrc.tile([128, 8 * 128], F32R, tag="w", name="w")
                        wtb = wsrc.tile([128, 8 * 128], F32R, tag="w", name="w")
                        for j in range(8):
                            nc.sync.dma_start(
                                wta[:, j * 128:(j + 1) * 128],
                                a_t[j * 128:(j + 1) * 128,
                                    blk * 128:(blk + 1) * 128])
                            nc.sync.dma_start(
                                wtb[:, j * 128:(j + 1) * 128],
                                b_t[j * 128:(j + 1) * 128,
                                    blk * 128:(blk + 1) * 128])
                        was = [wta[:, j * 128:(j + 1) * 128] for j in range(8)]
                        wbs = [wtb[:, j * 128:(j + 1) * 128] for j in range(8)]
                    else:
                        was = [a_t[j][:, blk * 128:(blk + 1) * 128] for j in range(8)]
                        wbs = [b_t[j][:, blk * 128:(blk + 1) * 128] for j in range(8)]
                    pP = [pmm.tile([128, 512], F32, tag="pp", name="pp") for _ in range(2)]
                    pN = [pmm.tile([128, 512], F32, tag="pn", name="pn") for _ in range(2)]
                    pC = [pmc.tile([128, 512], F32, tag="pc", name="pc") for _ in range(2)]
                    for j in range(8):
                        st, sp = (j == 0), (j == 7)
                        for hh in range(2):
                            nc.tensor.matmul(pP[hh][:], was[j], rhs_cs(CT, j, hh),
                                             start=st, stop=sp)
                            nc.tensor.matmul(pC[hh][:], was[j], rhs_cs(ST, j, hh),
                                             start=st, stop=False)
                        for hh in range(2):
                            nc.tensor.matmul(pN[hh][:], wbs[j], rhs_cs(ST, j, hh),
                                             start=st, stop=sp)
                            nc.tensor.matmul(pC[hh][:], wbs[j], rhs_cs(CT, j, hh),
                                             start=False, stop=sp)
                    for hh in range(2):
                        nt = ttp.tile([128, 512], F32, tag="tt", name="tt")
                        SC.copy(nt[:], pN[hh][:])
                        if dst is not None:
                            dr, di = dst
                            V.tensor_sub(dr[blk][:, hh * 512:(hh + 1) * 512],
                                         pP[hh][:], nt[:])
                            SC.mul(di[blk][:, hh * 512:(hh + 1) * 512],
                                   pC[hh][:], sigma)
                        else:
                            rt = th("s2r")
                            V.tensor_sub(rt[:], pP[hh][:], nt[:])
                            it = th("s2r")
                            SC.mul(it[:], pC[hh][:], sigma)
                            out_emit(blk, hh, rt, it)

            def fft_fwd(src, emit, jorder):
                t1r = [t1t.tile([128, D], F32R, tag="t1r", name="t1r") for _ in range(8)]
                t1i = [t1t.tile([128, D], F32R, tag="t1i", name="t1i") for _ in range(8)]
                real_stage(src, t1r, t1i, jorder)
                cplx_stage(t1r, t1i, emit, -1.0)

            def emit_fz(blk, hh, rt, it):
                sth(dt_["FZR"], blk, hh, rt)
                sth(dt_["FZI"], blk, hh, it)

            # combine fused into fwd(SH)'s stage-2 emit: rt/it are the FS
            # chunk in SBUF; G = P1*FS + P2*FZ written straight to GR/GI.
            def emit_combine(blk, hh, rt, it):
                fzr = ldh(dt_["FZR"], blk, hh, "l1")
                fzi = ldh(dt_["FZI"], blk, hh, "l2")
                f1r = ldh(p1r_p, blk, hh, "l3")
                f1i = ldh(p1i_p, blk, hh, "l4")
                f2r = ldh(p2r_p, blk, hh, "l6")
                f2i = ldh(p2i_p, blk, hh, "m")

                t1 = th("p")
                t2 = th("ta")
                gr = th("z", F32R)
                V.tensor_mul(t1[:], f1r[:], rt[:])
                GP.tensor_tensor(t2[:], f1i[:], it[:], ALU.mult)
                V.tensor_sub(t1[:], t1[:], t2[:])
                t4 = th("tb")
                V.tensor_mul(t4[:], f2r[:], fzr[:])
                V.tensor_add(t1[:], t1[:], t4[:])
                GP.tensor_tensor(t2[:], f2i[:], fzi[:], ALU.mult)
                V.tensor_sub(gr[:], t1[:], t2[:])
                sth(dt_["GR"], blk, hh, gr)

                t3 = th("p")
                gi = th("u", F32R)
                V.tensor_mul(t3[:], f1r[:], it[:])
                GP.tensor_tensor(t2[:], f1i[:], rt[:], ALU.mult)
                V.tensor_add(t3[:], t3[:], t2[:])
                GP.tensor_tensor(t2[:], f2r[:], fzi[:], ALU.mult)
                V.tensor_add(t3[:], t3[:], t2[:])
                GP.tensor_tensor(t2[:], f2i[:], fzr[:], ALU.mult)
                V.tensor_add(gi[:], t3[:], t2[:])
                sth(dt_["GI"], blk, hh, gi)

            def fft_inv(srcR, srcI, dstR, dstI):
                wr = [t1t.tile([128, D], F32R, tag="t1r", name="t1r") for _ in range(8)]
                wi = [t1t.tile([128, D], F32R, tag="t1i", name="t1i") for _ in range(8)]
                cplx_stage(srcR, srcI, None, 1.0, from_dram=True, dst=(wr, wi))

                def emit2(blk, hh, rt, it):
                    nc.sync.dma_start(
                        dstR[blk * 128:(blk + 1) * 128, hh * 512:(hh + 1) * 512], rt[:])
                    nc.sync.dma_start(
                        dstI[blk * 128:(blk + 1) * 128, hh * 512:(hh + 1) * 512], it[:])
                cplx_stage(wr, wi, emit2, 1.0)

            # Rotated body: phase_fused leads, so it overlaps the previous
            # iteration's inverse-FFT matmuls and feeds fwd(Z) stage 1
            # chunk-by-chunk. With zero-init state, iteration 1's fused
            # reproduces the Z seed (VCEN*Ys in the center) exactly, and the
            # dropped trailing fused never affected X. Output unchanged.
            def body():
                phase_fused()
                fft_fwd(dt_["Z"], emit_fz, ORDER)
                phase_a2()
                fft_fwd(dt_["SH"], emit_combine, ORDER)
                fft_inv(dt_["GR"], dt_["GI"], dt_["X"], dt_["HX"])

            if use_loop and iters % 2 == 0:
                with tc.For_i(0, iters // 2, 1,
                              hint_engines=(mybir.EngineType.PE, mybir.EngineType.DVE,
                                            mybir.EngineType.Activation, mybir.EngineType.SP)):
                    body()
                    body()
            elif use_loop:
                with tc.For_i(0, iters, 1,
                              hint_engines=(mybir.EngineType.PE, mybir.EngineType.DVE,
                                            mybir.EngineType.Activation, mybir.EngineType.SP)):
                    body()
            else:
                for _ in range(iters):
                    body()

            for q in range(4):
                t = th("s2r")
                nc.sync.dma_start(
                    t[:], dt_["X"][256 + q * 128:256 + (q + 1) * 128, 256:768])
                nc.sync.dma_start(out_p[q * 128:(q + 1) * 128, :], t[:])
            if debug:
                for nm, dp in dbg_p.items():
                    for mi in range(8):
                        for hi in range(2):
                            t = th("s2r")
                            src = dt_[nm][mi * 128:(mi + 1) * 128,
                                          hi * 512:(hi + 1) * 512]
                            if dt_[nm].dtype == F32R:
                                src = src.bitcast(F32)
                            nc.sync.dma_start(t[:], src)
                            nc.sync.dma_start(
                                dp[mi * 128:(mi + 1) * 128,
                                   hi * 512:(hi + 1) * 512], t[:])

    nc.compile()
    return nc


# ----------------------------------------------------------------- host side

def _host_consts(h):
    h64 = np.asarray(h, dtype=np.float64)
    hp = np.pad(h64, 256)
    Hf = np.fft.fft2(np.fft.ifftshift(hp))
    lapl = np.zeros((D, D))
    lapl[0, 0] = 4.0
    lapl[0, 1] = -1.0
    lapl[1, 0] = -1.0
    lapl[0, -1] = -1.0
    lapl[-1, 0] = -1.0
    LtL = np.fft.fft2(lapl)
    Rdiv = 1.0 / (MU1 * np.abs(np.conj(Hf) * Hf) + MU2 * np.abs(LtL) + MU3)
    M = 1.0 + 1j * Hf / ALPHA
    P1 = M * MU3 * Rdiv / (D * D)
    P2 = M * MU1 * Rdiv * np.conj(Hf) / (D * D)
    n = np.arange(D)
    ang = 2.0 * np.pi * np.outer(n, n) / D
    return {
        "cmat": np.cos(ang).astype(np.float32),
        "smat": np.sin(ang).astype(np.float32),
        "idp": np.eye(128, dtype=np.float32),
        "p1r": np.ascontiguousarray(P1.real).astype(np.float32),
        "p1i": np.ascontiguousarray(P1.imag).astype(np.float32),
        "p2r": np.ascontiguousarray(P2.real).astype(np.float32),
        "p2i": np.ascontiguousarray(P2.imag).astype(np.float32),
    }


def kernel(y, h, iters=ITERS, use_loop=True, debug=False, raw=False):
    y = np.asarray(y)
    h = np.asarray(h)
    key = (iters, use_loop, debug)
    if key not in _CACHE:
        _CACHE[key] = _build(iters, use_loop, debug)
    nc = _CACHE[key]
    consts = _host_consts(h)
    in_maps = []
    for i in range(8):
        m = dict(consts)
        m["ys"] = (y[i, 0].astype(np.float64) / MU1).astype(np.float32)
        in_maps.append(m)
    res = run_bass_kernel_spmd(nc, in_maps, core_ids=list(range(8)))
    if raw:
        return res
    out = np.stack([res.results[i]["out"] for i in range(8)])[:, None]
    return out.astype(np.float32)



# revision 2
# speedup vs baseline: 1.0178x; 1.0178x over previous
"""ADMM-Net (DiffuserCam-style ADMM deconvolution) on 8 TRN2 NeuronCores.

Data-parallel: one 512x512 image per core, 20 ADMM iterations on the padded
1024x1024 grid. All fftshifts cancel algebraically; each iteration needs
2 real forward 2D-FFTs (of s_hat, Z) and 1 packed complex inverse 2D-FFT
that yields X' (real part) and H(X') (imag part) simultaneously. FFTs are
DFT-by-matmul on the TensorEngine in float16 (2 cols/cycle + fast weight
load). fp16 is safe here: all matmul operand sites have max ~0.7..300
(ALPHA packing keeps HX ~ X scale) except the G spectrum (~4e-4, subnormal
territory), which is scaled up by 2^24 (folded into host P1/P2) and
unscaled 2^-24 in the inverse-FFT drain ops. Validated vs fp64 reference
in sim: rel err 6.2e-3 (fp32: 3.9e-3, tolerance 2e-2).

Scaled state (A1=a1/MU1, A2=a2/MU2, A3=a3/MU3, HX=H(X) carried):
  W2d = soft(psi_d(X)+A2d, TAU/MU2) - A2d   [= psi_d(X) - clamp(q, +-thr)]
  T3  = max(A3+X,0) - A3                    [= X + relu(T3old - 2X)]
  u   = A1 + HX + pad(y)/MU1;  Z = MU1*Vdiv*u - A1
        (MU1*Vdiv = 1 outside center, MU1/(1+MU1) inside => Z=HX outside)
  s^  = T3 + psit(W2)
  G   = P1*F(s^) + P2*F(Z)   [P1=(1+iHf)*MU3*Rdiv, P2=(1+iHf)*MU1*Rdiv*conj(Hf),
                              1/N^2 inverse scale + 2^24 folded in]
  X' + i*HX' = IFFT2(G);  A1'=HX'-Z, A2d'=psi_d(X')-W2d, A3'=X'-T3
Output: X[256:768, 256:768] after 20 iterations.
"""
import numpy as np

import concourse.bass as bass
import concourse.bacc as bacc
import concourse.mybir as mybir
import concourse.tile as tile
from concourse.bass_utils import run_bass_kernel_spmd

F32 = mybir.dt.float32
F16 = mybir.dt.float16
AF = mybir.ActivationFunctionType
ALU = mybir.AluOpType

D = 1024
MU1, MU2, MU3 = 1e-6, 1e-5, 1e-5
TAU = 1e-4 * 1000.0
CTHR = TAU / MU2          # 1e4
VCEN = MU1 / (1.0 + MU1)  # center factor of MU1*Vdiv
ALPHA = 295.6  # ~rms|Hf| for h~U[0,1]: balances packed inverse (HX stored /ALPHA)
GS = float(2 ** 24)       # G-spectrum fp16 scale (unscaled in inverse drain)
GSI = 1.0 / GS
ITERS = 20

_CACHE = {}


def _build(iters=ITERS, use_loop=True, debug=False):
    nc = bacc.Bacc()

    ys_p = nc.declare_dram_parameter("ys", [512, 512], F32, isOutput=False)
    c_p = nc.declare_dram_parameter("cmat", [D, D], F16, isOutput=False)
    s_p = nc.declare_dram_parameter("smat", [D, D], F16, isOutput=False)
    p1r_p = nc.declare_dram_parameter("p1r", [D, D], F32, isOutput=False)
    p1i_p = nc.declare_dram_parameter("p1i", [D, D], F32, isOutput=False)
    p2r_p = nc.declare_dram_parameter("p2r", [D, D], F32, isOutput=False)
    p2i_p = nc.declare_dram_parameter("p2i", [D, D], F32, isOutput=False)
    out_p = nc.declare_dram_parameter("out", [512, 512], F32, isOutput=True)
    dbg_p = {}
    if debug:
        for nm in ("Z", "SH", "GR", "GI", "X", "HX"):
            dbg_p[nm] = nc.declare_dram_parameter("dbg_" + nm, [D, D], F32, isOutput=True)

    dt_ = {}
    for nm in ("X", "HX", "W20", "W21", "T3", "FZR", "FZI"):
        dt_[nm] = nc.dram_tensor(nm, [D, D], F32)
    for nm in ("SH", "Z", "GR", "GI"):
        dt_[nm] = nc.dram_tensor(nm, [D, D], F16)

    with tile.TileContext(nc) as tc:
        with (
            tc.tile_pool(name="consts", bufs=1) as cpool,
            tc.tile_pool(name="ld", bufs=2) as ld,     # loads (halves + xtf full)
            tc.tile_pool(name="tmp", bufs=2) as tmp,   # elementwise temps (halves)
            tc.tile_pool(name="wsrc", bufs=3) as wsrc,  # [128,128] lhsT data chunks
            tc.tile_pool(name="tt", bufs=2) as ttp,      # [128,512] drain temps
            tc.tile_pool(name="t1t", bufs=8) as t1t,     # inter-stage tiles
            tc.tile_pool(name="pmm", bufs=3, space="PSUM") as pmm,
            tc.tile_pool(name="pmc", bufs=2, space="PSUM") as pmc,
        ):
            CT = cpool.tile([128, 8 * D], F16, tag="CT", name="CT")
            ST = cpool.tile([128, 8 * D], F16, tag="ST", name="ST")
            nc.sync.dma_start(
                CT[:].rearrange("p (j k) -> p j k", j=8),
                c_p.rearrange("(j p) k -> p j k", p=128),
            )
            nc.sync.dma_start(
                ST[:].rearrange("p (j k) -> p j k", j=8),
                s_p.rearrange("(j p) k -> p j k", p=128),
            )

            V = nc.vector
            SC = nc.scalar
            GP = nc.gpsimd

            # ---- zero-init state ----
            zz = cpool.tile([128, 512], F32, tag="zz", name="zz")
            V.memset(zz[:], 0.0)
            zz16 = cpool.tile([128, 512], F16, tag="zz16", name="zz16")
            V.memset(zz16[:], 0.0)
            for nm in ("X", "HX", "W20", "W21", "T3", "Z"):
                for mi in range(8):
                    for hi2 in range(2):
                        nc.sync.dma_start(
                            dt_[nm][mi * 128:(mi + 1) * 128,
                                    hi2 * 512:(hi2 + 1) * 512],
                            zz16[:] if nm == "Z" else zz[:])

            def ldh(dram, mi, hi, tag, shift=0, dtype=F32, eng=None):
                """Load a [128,512] half-tile (rows mi*128 + shift, cols hi*512)."""
                t = ld.tile([128, 512], dtype, tag=tag, name=tag)
                eng = eng or nc.sync
                r0 = mi * 128 + shift
                cs = slice(hi * 512, (hi + 1) * 512)
                if shift == 0:
                    eng.dma_start(t[:], dram[r0:r0 + 128, cs])
                elif shift == -1:
                    if mi == 0:
                        eng.dma_start(t[0:1, :], dram[D - 1:D, cs])
                        eng.dma_start(t[1:128, :], dram[0:127, cs])
                    else:
                        eng.dma_start(t[:], dram[r0:r0 + 128, cs])
                elif shift == 1:
                    if mi == 7:
                        eng.dma_start(t[0:127, :], dram[r0:D, cs])
                        eng.dma_start(t[127:128, :], dram[0:1, cs])
                    else:
                        eng.dma_start(t[:], dram[r0:r0 + 128, cs])
                return t

            def ldf(dram, mi, tag="xtf"):
                t = ld.tile([128, D], F32, tag=tag, name=tag)
                nc.sync.dma_start(t[:], dram[mi * 128:(mi + 1) * 128, :])
                return t

            def sth(dram, mi, hi, t):
                nc.sync.dma_start(
                    dram[mi * 128:(mi + 1) * 128, hi * 512:(hi + 1) * 512], t[:])

            def th(tag, dtype=F32):
                return tmp.tile([128, 512], dtype, tag=tag, name=tag)

            # Chunk order: mi=0 needs the wrap row (1023) so it must run
            # last; 1..7 pipeline behind the producer's emit order.
            ORDER = [1, 2, 3, 4, 5, 6, 7, 0]

            # ------------- fused end-of-iteration sweep -------------
            # from (X', HX~', W2, T3, Z) compute next (W2, T3, Z) in place.
            def phase_fused():
                for mi in ORDER:
                    xtf = ldf(dt_["X"], mi)
                    for hi in range(2):
                        hs = hi * 512
                        xh = xtf[:, hs:hs + 512]
                        xup = ldh(dt_["X"], mi, hi, "l1", shift=-1)
                        hxt = ldh(dt_["HX"], mi, hi, "l2")
                        w0o = ldh(dt_["W20"], mi, hi, "l4")
                        w1o = ldh(dt_["W21"], mi, hi, "l6")
                        t3o = ldh(dt_["T3"], mi, hi, "m")

                        # Z' first: it gates the next forward FFT's stage 1.
                        # Z' = alpha*HX~' outside; center: VCEN*u' - alpha*HX~' + Z
                        z = th("z", F16)
                        V.tensor_scalar_mul(z[:], hxt[:], ALPHA)
                        if 2 <= mi <= 5:
                            zold = ldh(dt_["Z"], mi, hi, "l3", dtype=F16)
                            lo, hicol = (256, 512) if hi == 0 else (0, 256)
                            yt = ld.tile([128, 256], F32, tag="yt", name="yt")
                            yc0 = hi * 512 + lo - 256
                            nc.sync.dma_start(
                                yt[:],
                                ys_p[(mi - 2) * 128:(mi - 1) * 128, yc0:yc0 + 256])
                            uc = th("u")
                            V.scalar_tensor_tensor(
                                uc[:, 0:256], hxt[:, lo:hicol], 2.0 * ALPHA,
                                zold[:, lo:hicol], ALU.mult, ALU.subtract)
                            V.tensor_add(uc[:, 0:256], uc[:, 0:256], yt[:])
                            a1c = th("tb")
                            V.scalar_tensor_tensor(
                                a1c[:, 0:256], hxt[:, lo:hicol], ALPHA,
                                zold[:, lo:hicol], ALU.mult, ALU.subtract)
                            V.scalar_tensor_tensor(
                                z[:, lo:hicol], uc[:, 0:256], VCEN, a1c[:, 0:256],
                                ALU.mult, ALU.subtract)
                        sth(dt_["Z"], mi, hi, z)

                        # ch0: p = psi0; q = 2*p - w0o; w0' = p - clamp(q,+-thr)
                        p = th("p")
                        V.tensor_sub(p[:], xup[:], xh)
                        q = th("ta")
                        V.scalar_tensor_tensor(q[:], p[:], 2.0, w0o[:],
                                               ALU.mult, ALU.subtract)
                        cl = th("tb")
                        V.tensor_scalar(cl[:], q[:], -CTHR, CTHR, ALU.max, ALU.min)
                        ta = th("u")
                        V.tensor_sub(ta[:], p[:], cl[:])
                        sth(dt_["W20"], mi, hi, ta)

                        # ch1
                        p1 = th("p")
                        if hi == 0:
                            V.tensor_sub(p1[:, 1:], xtf[:, 0:511], xtf[:, 1:512])
                            V.tensor_sub(p1[:, 0:1], xtf[:, D - 1:D], xtf[:, 0:1])
                        else:
                            V.tensor_sub(p1[:], xtf[:, 511:1023], xtf[:, 512:1024])
                        q1 = th("ta")
                        V.scalar_tensor_tensor(q1[:], p1[:], 2.0, w1o[:],
                                               ALU.mult, ALU.subtract)
                        cl1 = th("tb")
                        V.tensor_scalar(cl1[:], q1[:], -CTHR, CTHR, ALU.max, ALU.min)
                        ta1 = th("u")
                        V.tensor_sub(ta1[:], p1[:], cl1[:])
                        sth(dt_["W21"], mi, hi, ta1)

                        # T3' = X + relu(T3 - 2X)   (spread across V/SC/GP)
                        m = th("ta")
                        V.scalar_tensor_tensor(m[:], xh, -2.0, t3o[:],
                                               ALU.mult, ALU.add)
                        r = th("tb")
                        SC.activation(r[:], m[:], AF.Relu)
                        m2 = th("u")
                        GP.tensor_tensor(m2[:], xh, r[:], ALU.add)
                        sth(dt_["T3"], mi, hi, m2)

            # ---------------- phase A2: s^ = T3 + psit(W2) ----------------
            # Entirely on GpSimd: keeps the Vector queue free for the FFT
            # psum drains, so SH is ready before fwd(SH) stage 1 needs it.
            def phase_a2():
                for mi in ORDER:
                    w1f = ldf(dt_["W21"], mi)
                    for hi in range(2):
                        w0d = ldh(dt_["W20"], mi, hi, "l1", shift=1)
                        w0 = ldh(dt_["W20"], mi, hi, "l2")
                        t3 = ldh(dt_["T3"], mi, hi, "l3")
                        dt0 = th("p")
                        GP.tensor_tensor(dt0[:], w0d[:], w0[:], ALU.subtract)
                        e = th("ta")
                        if hi == 0:
                            GP.tensor_tensor(e[:], w1f[:, 1:513],
                                             w1f[:, 0:512], ALU.subtract)
                        else:
                            GP.tensor_tensor(e[:, 0:511], w1f[:, 513:1024],
                                             w1f[:, 512:1023], ALU.subtract)
                            GP.tensor_tensor(e[:, 511:512], w1f[:, 0:1],
                                             w1f[:, D - 1:D], ALU.subtract)
                        GP.tensor_tensor(dt0[:], dt0[:], e[:], ALU.add)
                        sh = th("z", F16)
                        GP.tensor_tensor(sh[:], dt0[:], t3[:], ALU.add)
                        sth(dt_["SH"], mi, hi, sh)

            # ---------------- generic FFT pass (data-as-weights) ----------------
            # Every stage: out = lhsT.T @ rhs with lhsT = 128x128 DATA chunk
            # (reused for 4 matmuls) and rhs = resident C/S rows [128,512].
            # Each stage transposes its input, so two stages per transform
            # give the correct orientation. Spectra land as [k0, k1].

            def rhs_cs(mat, j, half):
                return mat[:, j * D + half * 512: j * D + (half + 1) * 512]

            def real_stage(src_dram, outa, outb, jorder):
                """t1r/t1i tiles [cblk][c(128part), k0(1024)]:
                outa[cblk] = (x.T C)[cblk], outb = (x.T S)[cblk].
                Weight chunks load per row-block j (split DMAs) and the PSUM
                accumulation follows jorder, so stage 1 consumes the
                producer's row-chunks as they land in DRAM."""
                for cblk in range(8):
                    wt = wsrc.tile([128, 8 * 128], F16, tag="w", name="w")
                    for j in jorder:
                        nc.sync.dma_start(
                            wt[:, j * 128:(j + 1) * 128],
                            src_dram[j * 128:(j + 1) * 128,
                                     cblk * 128:(cblk + 1) * 128])
                    ws = [wt[:, j * 128:(j + 1) * 128] for j in range(8)]
                    pA = [pmm.tile([128, 512], F32, tag="pp", name="pp") for _ in range(2)]
                    pB = [pmm.tile([128, 512], F32, tag="pn", name="pn") for _ in range(2)]
                    for idx, j in enumerate(jorder):
                        st, sp = (idx == 0), (idx == 7)
                        for hh in range(2):
                            nc.tensor.matmul(pA[hh][:], ws[j], rhs_cs(CT, j, hh),
                                             start=st, stop=sp)
                            nc.tensor.matmul(pB[hh][:], ws[j], rhs_cs(ST, j, hh),
                                             start=st, stop=sp)
                    for hh in range(2):
                        SC.copy(outa[cblk][:, hh * 512:(hh + 1) * 512], pA[hh][:])
                        SC.copy(outb[cblk][:, hh * 512:(hh + 1) * 512], pB[hh][:])

            def cplx_stage(a_t, b_t, out_emit, sigma, from_dram=False, dst=None,
                           out_scale=1.0):
                """inputs a,b: 8 tiles [128,1024] (SBUF) or dram tensors.
                Emits per (blk, half): R = out_scale*(a.T C - b.T S),
                I = out_scale*sigma*(b.T C + a.T S)
                via out_emit(blk, half, r_tile_sbuf, i_tile_sbuf)."""
                for blk in range(8):
                    if from_dram:
                        wta = wsrc.tile([128, 8 * 128], F16, tag="w", name="w")
                        wtb = wsrc.tile([128, 8 * 128], F16, tag="w", name="w")
                        for j in range(8):
                            nc.sync.dma_start(
                                wta[:, j * 128:(j + 1) * 128],
                                a_t[j * 128:(j + 1) * 128,
                                    blk * 128:(blk + 1) * 128])
                            nc.sync.dma_start(
                                wtb[:, j * 128:(j + 1) * 128],
                                b_t[j * 128:(j + 1) * 128,
                                    blk * 128:(blk + 1) * 128])
                        was = [wta[:, j * 128:(j + 1) * 128] for j in range(8)]
                        wbs = [wtb[:, j * 128:(j + 1) * 128] for j in range(8)]
                    else:
                        was = [a_t[j][:, blk * 128:(blk + 1) * 128] for j in range(8)]
                        wbs = [b_t[j][:, blk * 128:(blk + 1) * 128] for j in range(8)]
                    pP = [pmm.tile([128, 512], F32, tag="pp", name="pp") for _ in range(2)]
                    pN = [pmm.tile([128, 512], F32, tag="pn", name="pn") for _ in range(2)]
                    pC = [pmc.tile([128, 512], F32, tag="pc", name="pc") for _ in range(2)]
                    for j in range(8):
                        st, sp = (j == 0), (j == 7)
                        for hh in range(2):
                            nc.tensor.matmul(pP[hh][:], was[j], rhs_cs(CT, j, hh),
                                             start=st, stop=sp)
                            nc.tensor.matmul(pC[hh][:], was[j], rhs_cs(ST, j, hh),
                                             start=st, stop=False)
                        for hh in range(2):
                            nc.tensor.matmul(pN[hh][:], wbs[j], rhs_cs(ST, j, hh),
                                             start=st, stop=sp)
                            nc.tensor.matmul(pC[hh][:], wbs[j], rhs_cs(CT, j, hh),
                                             start=False, stop=sp)
                    for hh in range(2):
                        nt = ttp.tile([128, 512], F32, tag="tt", name="tt")
                        if dst is not None:
                            SC.copy(nt[:], pN[hh][:])
                            dr, di = dst
                            V.tensor_sub(dr[blk][:, hh * 512:(hh + 1) * 512],
                                         pP[hh][:], nt[:])
                            SC.mul(di[blk][:, hh * 512:(hh + 1) * 512],
                                   pC[hh][:], sigma)
                        elif out_scale == 1.0:
                            SC.copy(nt[:], pN[hh][:])
                            rt = th("s2r")
                            V.tensor_sub(rt[:], pP[hh][:], nt[:])
                            it = th("s2r")
                            SC.mul(it[:], pC[hh][:], sigma)
                            out_emit(blk, hh, rt, it)
                        else:
                            SC.mul(nt[:], pN[hh][:], out_scale)
                            rt = th("s2r")
                            V.scalar_tensor_tensor(rt[:], pP[hh][:], out_scale,
                                                   nt[:], ALU.mult, ALU.subtract)
                            it = th("s2r")
                            SC.mul(it[:], pC[hh][:], sigma * out_scale)
                            out_emit(blk, hh, rt, it)

            def fft_fwd(src, emit, jorder):
                t1r = [t1t.tile([128, D], F16, tag="t1r", name="t1r") for _ in range(8)]
                t1i = [t1t.tile([128, D], F16, tag="t1i", name="t1i") for _ in range(8)]
                real_stage(src, t1r, t1i, jorder)
                cplx_stage(t1r, t1i, emit, -1.0)

            def emit_fz(blk, hh, rt, it):
                sth(dt_["FZR"], blk, hh, rt)
                sth(dt_["FZI"], blk, hh, it)

            # combine fused into fwd(SH)'s stage-2 emit: rt/it are the FS
            # chunk in SBUF; G = P1*FS + P2*FZ written straight to GR/GI.
            def emit_combine(blk, hh, rt, it):
                fzr = ldh(dt_["FZR"], blk, hh, "l1")
                fzi = ldh(dt_["FZI"], blk, hh, "l2")
                f1r = ldh(p1r_p, blk, hh, "l3")
                f1i = ldh(p1i_p, blk, hh, "l4")
                f2r = ldh(p2r_p, blk, hh, "l6")
                f2i = ldh(p2i_p, blk, hh, "m")

                t1 = th("p")
                t2 = th("ta")
                gr = th("z", F16)
                V.tensor_mul(t1[:], f1r[:], rt[:])
                GP.tensor_tensor(t2[:], f1i[:], it[:], ALU.mult)
                V.tensor_sub(t1[:], t1[:], t2[:])
                t4 = th("tb")
                V.tensor_mul(t4[:], f2r[:], fzr[:])
                V.tensor_add(t1[:], t1[:], t4[:])
                GP.tensor_tensor(t2[:], f2i[:], fzi[:], ALU.mult)
                V.tensor_sub(gr[:], t1[:], t2[:])
                sth(dt_["GR"], blk, hh, gr)

                t3 = th("p")
                gi = th("u", F16)
                V.tensor_mul(t3[:], f1r[:], it[:])
                GP.tensor_tensor(t2[:], f1i[:], rt[:], ALU.mult)
                V.tensor_add(t3[:], t3[:], t2[:])
                GP.tensor_tensor(t2[:], f2r[:], fzi[:], ALU.mult)
                V.tensor_add(t3[:], t3[:], t2[:])
                GP.tensor_tensor(t2[:], f2i[:], fzr[:], ALU.mult)
                V.tensor_add(gi[:], t3[:], t2[:])
                sth(dt_["GI"], blk, hh, gi)

            def fft_inv(srcR, srcI, dstR, dstI):
                wr = [t1t.tile([128, D], F16, tag="t1r", name="t1r") for _ in range(8)]
                wi = [t1t.tile([128, D], F16, tag="t1i", name="t1i") for _ in range(8)]
                cplx_stage(srcR, srcI, None, 1.0, from_dram=True, dst=(wr, wi))

                def emit2(blk, hh, rt, it):
                    nc.sync.dma_start(
                        dstR[blk * 128:(blk + 1) * 128, hh * 512:(hh + 1) * 512], rt[:])
                    nc.sync.dma_start(
                        dstI[blk * 128:(blk + 1) * 128, hh * 512:(hh + 1) * 512], it[:])
                cplx_stage(wr, wi, emit2, 1.0, out_scale=GSI)

            # Rotated body: phase_fused leads, so it overlaps the previous
            # iteration's inverse-FFT matmuls and feeds fwd(Z) stage 1
            # chunk-by-chunk. With zero-init state, iteration 1's fused
            # reproduces the Z seed (VCEN*Ys in the center) exactly, and the
            # dropped trailing fused never affected X. Output unchanged.
            def body():
                phase_fused()
                fft_fwd(dt_["Z"], emit_fz, ORDER)
                phase_a2()
                fft_fwd(dt_["SH"], emit_combine, ORDER)
                fft_inv(dt_["GR"], dt_["GI"], dt_["X"], dt_["HX"])

            if use_loop and iters % 2 == 0:
                with tc.For_i(0, iters // 2, 1,
                              hint_engines=(mybir.EngineType.PE, mybir.EngineType.DVE,
                                            mybir.EngineType.Activation, mybir.EngineType.SP)):
                    body()
                    body()
            elif use_loop:
                with tc.For_i(0, iters, 1,
                              hint_engines=(mybir.EngineType.PE, mybir.EngineType.DVE,
                                            mybir.EngineType.Activation, mybir.EngineType.SP)):
                    body()
            else:
                for _ in range(iters):
                    body()

            for q in range(4):
                t = th("s2r")
                nc.sync.dma_start(
                    t[:], dt_["X"][256 + q * 128:256 + (q + 1) * 128, 256:768])
                nc.sync.dma_start(out_p[q * 128:(q + 1) * 128, :], t[:])
            if debug:
                for nm, dp in dbg_p.items():
                    for mi in range(8):
                        for hi in range(2):
                            t = th("s2r")
                            src = dt_[nm][mi * 128:(mi + 1) * 128,
                                          hi * 512:(hi + 1) * 512]
                            nc.sync.dma_start(t[:], src)
                            nc.sync.dma_start(
                                dp[mi * 128:(mi + 1) * 128,
                                   hi * 512:(hi + 1) * 512], t[:])

    nc.compile()
    return nc


# ----------------------------------------------------------------- host side

def _host_consts(h):
    h64 = np.asarray(h, dtype=np.float64)
    hp = np.pad(h64, 256)
    Hf = np.fft.fft2(np.fft.ifftshift(hp))
    lapl = np.zeros((D, D))
    lapl[0, 0] = 4.0
    lapl[0, 1] = -1.0
    lapl[1, 0] = -1.0
    lapl[0, -1] = -1.0
    lapl[-1, 0] = -1.0
    LtL = np.fft.fft2(lapl)
    Rdiv = 1.0 / (MU1 * np.abs(np.conj(Hf) * Hf) + MU2 * np.abs(LtL) + MU3)
    M = 1.0 + 1j * Hf / ALPHA
    P1 = M * MU3 * Rdiv / (D * D) * GS
    P2 = M * MU1 * Rdiv * np.conj(Hf) / (D * D) * GS
    n = np.arange(D)
    ang = 2.0 * np.pi * np.outer(n, n) / D
    return {
        "cmat": np.cos(ang).astype(np.float16),
        "smat": np.sin(ang).astype(np.float16),
        "p1r": np.ascontiguousarray(P1.real).astype(np.float32),
        "p1i": np.ascontiguousarray(P1.imag).astype(np.float32),
        "p2r": np.ascontiguousarray(P2.real).astype(np.float32),
        "p2i": np.ascontiguousarray(P2.imag).astype(np.float32),
    }


def kernel(y, h, iters=ITERS, use_loop=True, debug=False, raw=False):
    y = np.asarray(y)
    h = np.asarray(h)
    key = (iters, use_loop, debug)
    if key not in _CACHE:
        _CACHE[key] = _build(iters, use_loop, debug)
    nc = _CACHE[key]
    consts = _host_consts(h)
    in_maps = []
    for i in range(8):
        m = dict(consts)
        m["ys"] = (y[i, 0].astype(np.float64) / MU1).astype(np.float32)
        in_maps.append(m)
    res = run_bass_kernel_spmd(nc, in_maps, core_ids=list(range(8)))
    if raw:
        return res
    out = np.stack([res.results[i]["out"] for i in range(8)])[:, None]
    return out.astype(np.float32)


# revision 10
# speedup vs baseline: 1.2091x; 1.1880x over previous
"""ADMM-Net (DiffuserCam-style ADMM deconvolution) on 8 TRN2 NeuronCores.

Data-parallel: one 512x512 image per core, 20 ADMM iterations on the padded
1024x1024 grid. All fftshifts cancel algebraically; each iteration needs
2 real forward 2D-FFTs (of s_hat, Z) and 1 packed complex inverse 2D-FFT
that yields X' (real part) and H(X') (imag part) simultaneously. FFTs are
DFT-by-matmul on the TensorEngine in float16.

v3: Z / SH / G spectra live in SBUF (no weight DMA, no DRAM round trip);
HX/W2/T3 state and FZ spectra are fp16 in DRAM; P1/P2 constants fp16.
The G spectrum (~4e-4, fp16-subnormal territory) is scaled by 2^24 (folded
into P1/P2) and unscaled in the inverse drain; FZ is drained at 1/8
(compensated in P2). PSUM-reading drains at the end of the inverse FFT run
on Scalar+GpSimd so the next iteration's fused elementwise phase (DVE)
overlaps the inverse-FFT matmuls. Validated in numpy sim: rel 7.7e-3
(fp32 3.9e-3, tolerance 2e-2).

Scaled state (A1=a1/MU1, A2=a2/MU2, A3=a3/MU3, HX=H(X) carried):
  W2d = soft(psi_d(X)+A2d, TAU/MU2) - A2d   [= psi_d(X) - clamp(q, +-thr)]
  T3  = max(A3+X,0) - A3                    [= X + relu(T3old - 2X)]
  u   = A1 + HX + pad(y)/MU1;  Z = MU1*Vdiv*u - A1
        (MU1*Vdiv = 1 outside center, MU1/(1+MU1) inside => Z=HX outside)
  s^  = T3 + psit(W2)
  G   = P1*F(s^) + P2*F(Z)   [P1=(1+iHf)*MU3*Rdiv, P2=(1+iHf)*MU1*Rdiv*conj(Hf),
                              1/N^2 inverse scale + 2^24 folded in]
  X' + i*HX' = IFFT2(G);  A1'=HX'-Z, A2d'=psi_d(X')-W2d, A3'=X'-T3
Output: X[256:768, 256:768] after 20 iterations.
"""
import numpy as np

import concourse.bass as bass
import concourse.bacc as bacc
import concourse.mybir as mybir
import concourse.tile as tile
from concourse.bass_utils import run_bass_kernel_spmd

F32 = mybir.dt.float32
F16 = mybir.dt.float16
AF = mybir.ActivationFunctionType
ALU = mybir.AluOpType

D = 1024
MU1, MU2, MU3 = 1e-6, 1e-5, 1e-5
TAU = 1e-4 * 1000.0
CTHR = TAU / MU2          # 1e4
VCEN = MU1 / (1.0 + MU1)  # center factor of MU1*Vdiv
ALPHA = 295.6  # ~rms|Hf| for h~U[0,1]: balances packed inverse (HX stored /ALPHA)
GS = float(2 ** 24)       # G-spectrum fp16 scale (unscaled in inverse drain)
GSI = 1.0 / GS
FZS = 0.125               # FZ fp16 drain scale (max |FZ| ~1.3e5)
FZSI = 1.0 / FZS
ITERS = 20

_CACHE = {}


def _build(iters=ITERS, use_loop=True, debug=False):
    nc = bacc.Bacc()

    ys_p = nc.declare_dram_parameter("ys", [512, 512], F32, isOutput=False)
    c_p = nc.declare_dram_parameter("cmat", [D, D], F16, isOutput=False)
    s_p = nc.declare_dram_parameter("smat", [D, D], F16, isOutput=False)
    p1r_p = nc.declare_dram_parameter("p1r", [D, D], F16, isOutput=False)
    p1i_p = nc.declare_dram_parameter("p1i", [D, D], F16, isOutput=False)
    p2r_p = nc.declare_dram_parameter("p2r", [D, D], F16, isOutput=False)
    p2i_p = nc.declare_dram_parameter("p2i", [D, D], F16, isOutput=False)
    out_p = nc.declare_dram_parameter("out", [512, 512], F32, isOutput=True)

    dt_ = {"X": nc.dram_tensor("X", [D, D], F32)}
    for nm in ("HX", "W20", "W21", "T3", "FZR", "FZI"):
        dt_[nm] = nc.dram_tensor(nm, [D, D], F16)

    with tile.TileContext(nc) as tc:
        with (
            tc.tile_pool(name="consts", bufs=1) as cpool,
            tc.tile_pool(name="ld", bufs=2) as ld,     # loads (halves + xtf full)
            tc.tile_pool(name="tmp", bufs=2) as tmp,   # elementwise temps (halves)
            tc.tile_pool(name="tt", bufs=2) as ttp,      # [128,512] drain temps
            tc.tile_pool(name="t1t", bufs=8) as t1t,     # inter-stage tiles
            tc.tile_pool(name="pmm", bufs=3, space="PSUM") as pmm,
            tc.tile_pool(name="pmc", bufs=2, space="PSUM") as pmc,
        ):
            CT = cpool.tile([128, 8 * D], F16, tag="CT", name="CT")
            ST = cpool.tile([128, 8 * D], F16, tag="ST", name="ST")
            nc.sync.dma_start(
                CT[:].rearrange("p (j k) -> p j k", j=8),
                c_p.rearrange("(j p) k -> p j k", p=128),
            )
            nc.sync.dma_start(
                ST[:].rearrange("p (j k) -> p j k", j=8),
                s_p.rearrange("(j p) k -> p j k", p=128),
            )

            V = nc.vector
            SC = nc.scalar
            GP = nc.gpsimd

            # ---- persistent SBUF state: Z, SH tiles [mi][hi], G [blk] ----
            zbuf = [[cpool.tile([128, 512], F16, tag=f"zb{mi}_{hi}",
                                name=f"zb{mi}_{hi}") for hi in range(2)]
                    for mi in range(8)]
            shb = [[cpool.tile([128, 512], F16, tag=f"sh{mi}_{hi}",
                               name=f"sh{mi}_{hi}") for hi in range(2)]
                   for mi in range(8)]
            gR = [cpool.tile([128, D], F16, tag=f"gr{b}", name=f"gr{b}")
                  for b in range(8)]
            gI = [cpool.tile([128, D], F16, tag=f"gi{b}", name=f"gi{b}")
                  for b in range(8)]

            # ---- zero-init state ----
            zz = cpool.tile([128, 512], F32, tag="zz", name="zz")
            V.memset(zz[:], 0.0)
            zz16 = cpool.tile([128, 512], F16, tag="zz16", name="zz16")
            V.memset(zz16[:], 0.0)
            for mi in range(8):
                for hi2 in range(2):
                    V.memset(zbuf[mi][hi2][:], 0.0)
                    nc.sync.dma_start(
                        dt_["X"][mi * 128:(mi + 1) * 128,
                                 hi2 * 512:(hi2 + 1) * 512], zz[:])
                    for nm in ("HX", "W20", "W21", "T3"):
                        nc.sync.dma_start(
                            dt_[nm][mi * 128:(mi + 1) * 128,
                                    hi2 * 512:(hi2 + 1) * 512], zz16[:])

            def ldh(dram, mi, hi, tag, shift=0, dtype=F32, eng=None):
                """Load a [128,512] half-tile (rows mi*128 + shift, cols hi*512)."""
                t = ld.tile([128, 512], dtype, tag=tag, name=tag)
                eng = eng or nc.sync
                r0 = mi * 128 + shift
                cs = slice(hi * 512, (hi + 1) * 512)
                if shift == 0:
                    eng.dma_start(t[:], dram[r0:r0 + 128, cs])
                elif shift == -1:
                    if mi == 0:
                        eng.dma_start(t[0:1, :], dram[D - 1:D, cs])
                        eng.dma_start(t[1:128, :], dram[0:127, cs])
                    else:
                        eng.dma_start(t[:], dram[r0:r0 + 128, cs])
                elif shift == 1:
                    if mi == 7:
                        eng.dma_start(t[0:127, :], dram[r0:D, cs])
                        eng.dma_start(t[127:128, :], dram[0:1, cs])
                    else:
                        eng.dma_start(t[:], dram[r0:r0 + 128, cs])
                return t

            def ldf(dram, mi, tag="xtf"):
                t = ld.tile([128, D], F32, tag=tag, name=tag)
                nc.sync.dma_start(t[:], dram[mi * 128:(mi + 1) * 128, :])
                return t

            def sth(dram, mi, hi, t):
                nc.sync.dma_start(
                    dram[mi * 128:(mi + 1) * 128, hi * 512:(hi + 1) * 512], t[:])

            def th(tag, dtype=F32):
                return tmp.tile([128, 512], dtype, tag=tag, name=tag)

            # Chunk order: mi=0 needs the wrap row (1023) so it must run
            # last; 1..7 pipeline behind the producer's emit order.
            ORDER = [1, 2, 3, 4, 5, 6, 7, 0]

            # ------------- fused end-of-iteration sweep -------------
            # from (X', HX~', W2, T3, Z) compute next (W2, T3, Z) in place.
            # DVE + light SC/GP; no PSUM deps => overlaps the inverse FFT.
            def phase_fused():
                for mi in ORDER:
                    xtf = ldf(dt_["X"], mi)
                    for hi in range(2):
                        hs = hi * 512
                        xh = xtf[:, hs:hs + 512]
                        xup = ldh(dt_["X"], mi, hi, "l1", shift=-1)
                        hxt = ldh(dt_["HX"], mi, hi, "l2", dtype=F16)
                        w0o = ldh(dt_["W20"], mi, hi, "l4", dtype=F16)
                        w1o = ldh(dt_["W21"], mi, hi, "l6", dtype=F16)
                        t3o = ldh(dt_["T3"], mi, hi, "m", dtype=F16)

                        # Z' first: it gates the next forward FFT's stage 1.
                        # center block: read old Z from zbuf, then overwrite.
                        z = zbuf[mi][hi]
                        if 2 <= mi <= 5:
                            lo, hicol = (256, 512) if hi == 0 else (0, 256)
                            zold = z[:, lo:hicol]
                            yt = ld.tile([128, 256], F32, tag="yt", name="yt")
                            yc0 = hi * 512 + lo - 256
                            nc.sync.dma_start(
                                yt[:],
                                ys_p[(mi - 2) * 128:(mi - 1) * 128, yc0:yc0 + 256])
                            uc = th("u")
                            V.scalar_tensor_tensor(
                                uc[:, 0:256], hxt[:, lo:hicol], 2.0 * ALPHA,
                                zold, ALU.mult, ALU.subtract)
                            V.tensor_add(uc[:, 0:256], uc[:, 0:256], yt[:])
                            a1c = th("tb")
                            V.scalar_tensor_tensor(
                                a1c[:, 0:256], hxt[:, lo:hicol], ALPHA,
                                zold, ALU.mult, ALU.subtract)
                            zc = th("p")
                            V.scalar_tensor_tensor(
                                zc[:, 0:256], uc[:, 0:256], VCEN, a1c[:, 0:256],
                                ALU.mult, ALU.subtract)
                            V.tensor_scalar_mul(z[:], hxt[:], ALPHA)
                            V.tensor_copy(z[:, lo:hicol], zc[:, 0:256])
                        else:
                            V.tensor_scalar_mul(z[:], hxt[:], ALPHA)

                        # ch0: p = psi0; q = 2*p - w0o; w0' = p - clamp(q,+-thr)
                        p = th("p")
                        V.tensor_sub(p[:], xup[:], xh)
                        q = th("ta")
                        V.scalar_tensor_tensor(q[:], p[:], 2.0, w0o[:],
                                               ALU.mult, ALU.subtract)
                        cl = th("tb")
                        V.tensor_scalar(cl[:], q[:], -CTHR, CTHR, ALU.max, ALU.min)
                        ta = th("u", F16)
                        V.tensor_sub(ta[:], p[:], cl[:])
                        sth(dt_["W20"], mi, hi, ta)

                        # ch1
                        p1 = th("p")
                        if hi == 0:
                            V.tensor_sub(p1[:, 1:], xtf[:, 0:511], xtf[:, 1:512])
                            V.tensor_sub(p1[:, 0:1], xtf[:, D - 1:D], xtf[:, 0:1])
                        else:
                            V.tensor_sub(p1[:], xtf[:, 511:1023], xtf[:, 512:1024])
                        q1 = th("ta")
                        V.scalar_tensor_tensor(q1[:], p1[:], 2.0, w1o[:],
                                               ALU.mult, ALU.subtract)
                        cl1 = th("tb")
                        V.tensor_scalar(cl1[:], q1[:], -CTHR, CTHR, ALU.max, ALU.min)
                        ta1 = th("u", F16)
                        V.tensor_sub(ta1[:], p1[:], cl1[:])
                        sth(dt_["W21"], mi, hi, ta1)

                        # T3' = X + relu(T3 - 2X)   (V -> SC -> GP chain)
                        m = th("ta")
                        V.scalar_tensor_tensor(m[:], xh, -2.0, t3o[:],
                                               ALU.mult, ALU.add)
                        r = th("tb")
                        SC.activation(r[:], m[:], AF.Relu)
                        m2 = th("m2", F16)
                        GP.tensor_tensor(m2[:], xh, r[:], ALU.add)
                        sth(dt_["T3"], mi, hi, m2)

            # ---------------- phase A2: s^ = T3 + psit(W2) ----------------
            # mostly GpSimd; col-shift diff on DVE for balance.
            def phase_a2():
                for mi in ORDER:
                    w1f = ldf16(dt_["W21"], mi, tag="w1f")
                    for hi in range(2):
                        w0d = ldh(dt_["W20"], mi, hi, "l1", shift=1, dtype=F16)
                        w0 = ldh(dt_["W20"], mi, hi, "l2", dtype=F16)
                        t3 = ldh(dt_["T3"], mi, hi, "l3", dtype=F16)
                        dt0 = th("p")
                        GP.tensor_tensor(dt0[:], w0d[:], w0[:], ALU.subtract)
                        e = th("ta")
                        if hi == 0:
                            V.tensor_sub(e[:], w1f[:, 1:513], w1f[:, 0:512])
                        else:
                            V.tensor_sub(e[:, 0:511], w1f[:, 513:1024],
                                         w1f[:, 512:1023])
                            V.tensor_sub(e[:, 511:512], w1f[:, 0:1],
                                         w1f[:, D - 1:D])
                        GP.tensor_tensor(dt0[:], dt0[:], e[:], ALU.add)
                        GP.tensor_tensor(shb[mi][hi][:], dt0[:], t3[:], ALU.add)

            # w1f must be F16-typed full-row load
            def ldf16(dram, mi, tag):
                t = ld.tile([128, D], F16, tag=tag, name=tag)
                nc.sync.dma_start(t[:], dram[mi * 128:(mi + 1) * 128, :])
                return t

            # ---------------- generic FFT pass (data-as-weights) ----------------
            # Every stage: out = lhsT.T @ rhs with lhsT = 128x128 DATA chunk
            # (reused for 4 matmuls) and rhs = resident C/S rows [128,512].
            # Each stage transposes its input, so two stages per transform
            # give the correct orientation. Spectra land as [k0, k1].

            def rhs_cs(mat, j, half):
                return mat[:, j * D + half * 512: j * D + (half + 1) * 512]

            def real_stage(srcbuf, outa, outb, jorder):
                """srcbuf: [mi][hi] SBUF tiles. t1r/t1i [cblk][c, k0]:
                outa[cblk] = (x.T C)[cblk], outb = (x.T S)[cblk]. PSUM
                accumulation follows jorder = producer's emit order."""
                for cblk in range(8):
                    hi, qd = cblk // 4, cblk % 4
                    ws = [srcbuf[j][hi][:, qd * 128:(qd + 1) * 128]
                          for j in range(8)]
                    pA = [pmm.tile([128, 512], F32, tag="pp", name="pp") for _ in range(2)]
                    pB = [pmm.tile([128, 512], F32, tag="pn", name="pn") for _ in range(2)]
                    for idx, j in enumerate(jorder):
                        st, sp = (idx == 0), (idx == 7)
                        for hh in range(2):
                            nc.tensor.matmul(pA[hh][:], ws[j], rhs_cs(CT, j, hh),
                                             start=st, stop=sp)
                            nc.tensor.matmul(pB[hh][:], ws[j], rhs_cs(ST, j, hh),
                                             start=st, stop=sp)
                    for hh in range(2):
                        SC.copy(outa[cblk][:, hh * 512:(hh + 1) * 512], pA[hh][:])
                        SC.copy(outb[cblk][:, hh * 512:(hh + 1) * 512], pB[hh][:])

            def cplx_stage(a_t, b_t, out_emit, sigma, dst=None, tail_drain=False,
                           out_scale=1.0, out_dtype=F32):
                """inputs a,b: 8 tiles [128,1024] (SBUF).
                Emits per (blk, half): R = s*(a.T C - b.T S),
                I = s*sigma*(b.T C + a.T S). dst=(wr,wi): write SBUF tiles.
                tail_drain: PSUM-read ops on SC only + GP combine, keeping
                DVE free (used for the last stage of the inverse FFT)."""
                for blk in range(8):
                    was = [a_t[j][:, blk * 128:(blk + 1) * 128] for j in range(8)]
                    wbs = [b_t[j][:, blk * 128:(blk + 1) * 128] for j in range(8)]
                    pP = [pmm.tile([128, 512], F32, tag="pp", name="pp") for _ in range(2)]
                    pN = [pmm.tile([128, 512], F32, tag="pn", name="pn") for _ in range(2)]
                    pC = [pmc.tile([128, 512], F32, tag="pc", name="pc") for _ in range(2)]
                    for j in range(8):
                        st, sp = (j == 0), (j == 7)
                        for hh in range(2):
                            nc.tensor.matmul(pP[hh][:], was[j], rhs_cs(CT, j, hh),
                                             start=st, stop=sp)
                            nc.tensor.matmul(pC[hh][:], was[j], rhs_cs(ST, j, hh),
                                             start=st, stop=False)
                        for hh in range(2):
                            nc.tensor.matmul(pN[hh][:], wbs[j], rhs_cs(ST, j, hh),
                                             start=st, stop=sp)
                            nc.tensor.matmul(pC[hh][:], wbs[j], rhs_cs(CT, j, hh),
                                             start=False, stop=sp)
                    for hh in range(2):
                        if dst is not None:
                            nt = ttp.tile([128, 512], F32, tag="tt", name="tt")
                            SC.copy(nt[:], pN[hh][:])
                            dr, di = dst
                            V.tensor_sub(dr[blk][:, hh * 512:(hh + 1) * 512],
                                         pP[hh][:], nt[:])
                            SC.mul(di[blk][:, hh * 512:(hh + 1) * 512],
                                   pC[hh][:], sigma)
                        elif tail_drain:
                            # SC reads PSUM, GP combines in SBUF (DVE-free)
                            aa = ttp.tile([128, 512], F32, tag="tt", name="tt")
                            bb = ttp.tile([128, 512], F32, tag="tb2", name="tb2")
                            SC.mul(aa[:], pP[hh][:], GSI)
                            SC.mul(bb[:], pN[hh][:], GSI)
                            rt = th("s2r")
                            GP.tensor_tensor(rt[:], aa[:], bb[:], ALU.subtract)
                            it = th("s2i", F16)
                            SC.mul(it[:], pC[hh][:], sigma * GSI)
                            out_emit(blk, hh, rt, it)
                        elif out_scale == 1.0:
                            nt = ttp.tile([128, 512], F32, tag="tt", name="tt")
                            SC.copy(nt[:], pN[hh][:])
                            rt = th("s2r")
                            V.tensor_sub(rt[:], pP[hh][:], nt[:])
                            it = th("s2i")
                            SC.mul(it[:], pC[hh][:], sigma)
                            out_emit(blk, hh, rt, it)
                        else:
                            nt = ttp.tile([128, 512], F32, tag="tt", name="tt")
                            SC.mul(nt[:], pN[hh][:], out_scale)
                            rt = th("s2r", out_dtype)
                            V.scalar_tensor_tensor(rt[:], pP[hh][:], out_scale,
                                                   nt[:], ALU.mult, ALU.subtract)
                            it = th("s2i", out_dtype)
                            SC.mul(it[:], pC[hh][:], sigma * out_scale)
                            out_emit(blk, hh, rt, it)

            def fft_fwd(src, emit, jorder, out_scale=1.0, out_dtype=F32):
                t1r = [t1t.tile([128, D], F16, tag="t1r", name="t1r") for _ in range(8)]
                t1i = [t1t.tile([128, D], F16, tag="t1i", name="t1i") for _ in range(8)]
                real_stage(src, t1r, t1i, jorder)
                cplx_stage(t1r, t1i, emit, -1.0, out_scale=out_scale,
                           out_dtype=out_dtype)

            # FZ drained at FZS scale into fp16 DRAM (P2 compensates)
            def emit_fz(blk, hh, rt, it):
                sth(dt_["FZR"], blk, hh, rt)
                sth(dt_["FZI"], blk, hh, it)

            # combine fused into fwd(SH)'s stage-2 emit: rt/it are the FS
            # chunk in SBUF; G = P1*FS + P2*FZ written straight to gR/gI.
            def emit_combine(blk, hh, rt, it):
                fzr = ldh(dt_["FZR"], blk, hh, "l1", dtype=F16)
                fzi = ldh(dt_["FZI"], blk, hh, "l2", dtype=F16)
                f1r = ldh(p1r_p, blk, hh, "l3", dtype=F16)
                f1i = ldh(p1i_p, blk, hh, "l4", dtype=F16)
                f2r = ldh(p2r_p, blk, hh, "l6", dtype=F16)
                f2i = ldh(p2i_p, blk, hh, "m", dtype=F16)

                cs = slice(hh * 512, (hh + 1) * 512)
                t1 = th("p")
                t2 = th("ta")
                V.tensor_mul(t1[:], f1r[:], rt[:])
                GP.tensor_tensor(t2[:], f1i[:], it[:], ALU.mult)
                V.tensor_sub(t1[:], t1[:], t2[:])
                t4 = th("tb")
                V.tensor_mul(t4[:], f2r[:], fzr[:])
                V.tensor_add(t1[:], t1[:], t4[:])
                GP.tensor_tensor(t2[:], f2i[:], fzi[:], ALU.mult)
                V.tensor_sub(gR[blk][:, cs], t1[:], t2[:])

                t3 = th("p")
                V.tensor_mul(t3[:], f1r[:], it[:])
                GP.tensor_tensor(t2[:], f1i[:], rt[:], ALU.mult)
                V.tensor_add(t3[:], t3[:], t2[:])
                GP.tensor_tensor(t2[:], f2r[:], fzi[:], ALU.mult)
                V.tensor_add(t3[:], t3[:], t2[:])
                GP.tensor_tensor(t2[:], f2i[:], fzr[:], ALU.mult)
                V.tensor_add(gI[blk][:, cs], t3[:], t2[:])

            def fft_inv(dstR, dstI):
                wr = [t1t.tile([128, D], F16, tag="t1r", name="t1r") for _ in range(8)]
                wi = [t1t.tile([128, D], F16, tag="t1i", name="t1i") for _ in range(8)]
                cplx_stage(gR, gI, None, 1.0, dst=(wr, wi))

                def emit2(blk, hh, rt, it):
                    nc.sync.dma_start(
                        dstR[blk * 128:(blk + 1) * 128, hh * 512:(hh + 1) * 512], rt[:])
                    nc.sync.dma_start(
                        dstI[blk * 128:(blk + 1) * 128, hh * 512:(hh + 1) * 512], it[:])
                cplx_stage(wr, wi, emit2, 1.0, tail_drain=True)

            # Rotated body: phase_fused leads, so it overlaps the previous
            # iteration's inverse-FFT matmuls and feeds fwd(Z) stage 1
            # chunk-by-chunk. With zero-init state, iteration 1's fused
            # reproduces the Z seed (VCEN*Ys in the center) exactly, and the
            # dropped trailing fused never affected X. Output unchanged.
            def body():
                phase_fused()
                fft_fwd(zbuf, emit_fz, ORDER, out_scale=FZS, out_dtype=F16)
                phase_a2()
                fft_fwd(shb, emit_combine, ORDER)
                fft_inv(dt_["X"], dt_["HX"])

            if use_loop and iters % 2 == 0:
                with tc.For_i(0, iters // 2, 1,
                              hint_engines=(mybir.EngineType.PE, mybir.EngineType.DVE,
                                            mybir.EngineType.Activation, mybir.EngineType.SP)):
                    body()
                    body()
            elif use_loop:
                with tc.For_i(0, iters, 1,
                              hint_engines=(mybir.EngineType.PE, mybir.EngineType.DVE,
                                            mybir.EngineType.Activation, mybir.EngineType.SP)):
                    body()
            else:
                for _ in range(iters):
                    body()

            for q in range(4):
                t = th("s2r")
                nc.sync.dma_start(
                    t[:], dt_["X"][256 + q * 128:256 + (q + 1) * 128, 256:768])
                nc.sync.dma_start(out_p[q * 128:(q + 1) * 128, :], t[:])

    nc.compile()
    return nc


# ----------------------------------------------------------------- host side

def _host_consts(h):
    h64 = np.asarray(h, dtype=np.float64)
    hp = np.pad(h64, 256)
    Hf = np.fft.fft2(np.fft.ifftshift(hp))
    lapl = np.zeros((D, D))
    lapl[0, 0] = 4.0
    lapl[0, 1] = -1.0
    lapl[1, 0] = -1.0
    lapl[0, -1] = -1.0
    lapl[-1, 0] = -1.0
    LtL = np.fft.fft2(lapl)
    Rdiv = 1.0 / (MU1 * np.abs(np.conj(Hf) * Hf) + MU2 * np.abs(LtL) + MU3)
    M = 1.0 + 1j * Hf / ALPHA
    P1 = M * MU3 * Rdiv / (D * D) * GS
    P2 = M * MU1 * Rdiv * np.conj(Hf) / (D * D) * GS * FZSI
    n = np.arange(D)
    ang = 2.0 * np.pi * np.outer(n, n) / D
    for nm, v in (("P1", P1), ("P2", P2)):
        mx = max(np.abs(v.real).max(), np.abs(v.imag).max())
        assert mx < 60000.0, f"{nm} overflows fp16: {mx}"
    return {
        "cmat": np.cos(ang).astype(np.float16),
        "smat": np.sin(ang).astype(np.float16),
        "p1r": np.ascontiguousarray(P1.real).astype(np.float16),
        "p1i": np.ascontiguousarray(P1.imag).astype(np.float16),
        "p2r": np.ascontiguousarray(P2.real).astype(np.float16),
        "p2i": np.ascontiguousarray(P2.imag).astype(np.float16),
    }


def kernel(y, h, iters=ITERS, use_loop=True, debug=False, raw=False):
    y = np.asarray(y)
    h = np.asarray(h)
    key = (iters, use_loop, debug)
    if key not in _CACHE:
        _CACHE[key] = _build(iters, use_loop, debug)
    nc = _CACHE[key]
    consts = _host_consts(h)
    in_maps = []
    for i in range(8):
        m = dict(consts)
        m["ys"] = (y[i, 0].astype(np.float64) / MU1).astype(np.float32)
        in_maps.append(m)
    res = run_bass_kernel_spmd(nc, in_maps, core_ids=list(range(8)))
    if raw:
        return res
    out = np.stack([res.results[i]["out"] for i in range(8)])[:, None]
    return out.astype(np.float32)
